# revision 1
# baseline (speedup 1.0000x reference)
"""Sharded causal attention kernel for trn2 (per-core program builder), v2.

Sharding: 8 cores = 2 batches x 4 head-groups (4 heads each).
v2 structure:
  - projections run on RAW x tiles as they stream in (rmsnorm scale is
    deferred: q/k scaled post-rotary along free dim, v scaled per-partition)
  - attention emits head-pair-adjacent matmuls (PE row/col tile packing)
  - output projection interleaved per q-block
"""

from contextlib import ExitStack

import numpy as np

import concourse.bass as bass
import concourse.mybir as mybir
import concourse.tile as tile
from concourse import bacc
from concourse.bass import _add_dep_helper as add_dep

f32 = mybir.dt.float32
f32r = mybir.dt.float32r
bf16 = mybir.dt.bfloat16
AF = mybir.ActivationFunctionType
OP = mybir.AluOpType

D = 1024
HPC = 4
DH = 64
ROT = 32
P = 128
EPS = 1e-8
NEG = -1e30


def build_program(n=2048, mm_dt="f32r", use_kmask=False, dbg=False):
    KT = D // P
    NQB = n // 512
    NTOK = n // P
    NCH = n // 512
    mdt = {"f32": f32, "f32r": f32r, "bf16": bf16}[mm_dt]
    nc = bacc.Bacc("TRN2", target_bir_lowering=False, debug=False)

    def din(name, shape, dt_):
        return nc.dram_tensor(name, shape, dt_, kind="ExternalInput")

    xT_d = din("xT", [D, n], mdt)
    wq_d = din("wq", [D, HPC * DH], mdt)
    wk_d = din("wk", [D, HPC * DH], mdt)
    wv_d = din("wv", [D, HPC * DH], mdt)
    wqr_d = din("wqr", [D, 2 * P], mdt)   # [h0r|0|h1r|0 , h2r|0|h3r|0]
    wkr_d = din("wkr", [D, 2 * P], mdt)
    wo_d = din("wo", [HPC * DH, D], mdt)
    cos_d = din("cos128", [P, n], f32)
    sin_d = din("sin128", [P, n], f32)
    tri_d = din("tri", [P, P], f32)
    id_d = din("ident", [P, P], f32)
    km_d = din("kmask", [P, NTOK], f32) if use_kmask else None
    out_d = nc.dram_tensor("out", [n, D], f32, kind="ExternalOutput")
    dbg_d = {}
    if dbg:
        for nm, shp in (("dqT0", [P, n]), ("dqT1", [P, n]), ("dkT0", [P, n]),
                        ("dv0", [P, HPC * (DH + 1)]), ("dden", [HPC, n]),
                        ("dattn0", [P, n]), ("drs", [1, n])):
            dbg_d[nm] = nc.dram_tensor(nm, shp, f32, kind="ExternalOutput")
        dbg_d["dpv"] = nc.dram_tensor("dpv", [P, 512], f32, kind="ExternalOutput")
        dbg_d["dbcd"] = nc.dram_tensor("dbcd", [64, 512], f32, kind="ExternalOutput")

    with tile.TileContext(nc) as tc, ExitStack() as top:
        persist = top.enter_context(tc.tile_pool(name="persist", bufs=1))
        ones_f32 = persist.tile([P, 1], f32, name="ones_f32")
        nc.vector.memset(ones_f32, 1.0)
        ones_col = persist.tile([P, 1], mdt, name="ones_col")
        nc.vector.tensor_copy(ones_col, ones_f32)
        ones_row = persist.tile([1, P], f32, name="ones_row")
        nc.vector.memset(ones_row, 1.0)
        tri_sb = persist.tile([P, P], f32, name="tri_sb")
        nc.sync.dma_start(out=tri_sb, in_=tri_d[:])
        ident_sb = persist.tile([P, P], f32, name="ident_sb")
        nc.sync.dma_start(out=ident_sb, in_=id_d[:])
        if use_kmask:
            km_sb = persist.tile([P, NTOK], f32, name="km_sb")
            nc.sync.dma_start(out=km_sb, in_=km_d[:])

        qkv = top.enter_context(tc.tile_pool(name="qkv", bufs=1))
        qT = [qkv.tile([P, n], mdt, name=f"qT{m}", tag=f"qT{m}") for m in range(2)]
        kT = [qkv.tile([P, n], mdt, name=f"kT{m}", tag=f"kT{m}") for m in range(2)]
        v_sb = [qkv.tile([P, HPC * (DH + 1)], mdt, name=f"v{tk}", tag=f"v{tk}")
                for tk in range(NTOK)]
        normk = top.enter_context(tc.tile_pool(name="normk", bufs=1))
        rs_col = normk.tile([P, NTOK], f32, name="rs_col")
        # per-q-block attention output chunks (freed after their out-proj)
        late = top.enter_context(tc.tile_pool(name="late", bufs=1))
        if dbg:
            den_sb = [normk.tile([1, n], f32, name=f"den{h}", tag=f"den{h}")
                      for h in range(HPC)]
        wop = top.enter_context(tc.tile_pool(name="wop", bufs=1))

        with ExitStack() as phase_a:
            big = phase_a.enter_context(tc.tile_pool(name="big", bufs=1))
            x_sb = [big.tile([P, n], mdt, name=f"x{t}", tag=f"x{t}") for t in range(KT)]
            for t in range(KT):
                nc.sync.dma_start(out=x_sb[t], in_=xT_d[t * P:(t + 1) * P, :])
            wq, wk, wv, wqr, wkr = [], [], [], [], []
            for t in range(KT):
                for lst, dsrc, w_, nm in (
                        (wq, wq_d, HPC * DH, "wq"), (wk, wk_d, HPC * DH, "wk"),
                        (wv, wv_d, HPC * DH, "wv"), (wqr, wqr_d, 2 * P, "wqr"),
                        (wkr, wkr_d, 2 * P, "wkr")):
                    tl = big.tile([P, w_], mdt, name=f"{nm}{t}", tag=f"{nm}{t}")
                    nc.sync.dma_start(out=tl, in_=dsrc[t * P:(t + 1) * P, :])
                    lst.append(tl)

            rot = phase_a.enter_context(tc.tile_pool(name="rot", bufs=1))
            cos_sb = rot.tile([P, n], f32, name="cos_sb")
            sin_sb = rot.tile([P, n], f32, name="sin_sb")

            # ---- rmsnorm scale (overlaps the projection matmuls below) ----
            last_rs_inst = None
            with tc.tile_pool(name="normt", bufs=1) as normt, \
                 tc.tile_pool(name="sqp", bufs=2) as sqp, \
                 tc.tile_pool(name="pnorm", bufs=1, space="PSUM") as pnorm, \
                 tc.tile_pool(name="pbc", bufs=2, space="PSUM") as pbc:
                ssq = [pnorm.tile([1, 512], f32, name=f"ssq{c}", tag=f"ssq{c}")
                       for c in range(NCH)]
                for t in range(KT):
                    for c in range(NCH):
                        sq = sqp.tile([P, 512], mdt, name=f"sq{t}_{c}", tag="sq")
                        nc.vector.tensor_mul(sq, x_sb[t][:, c * 512:(c + 1) * 512],
                                             x_sb[t][:, c * 512:(c + 1) * 512])
                        nc.tensor.matmul(ssq[c], ones_col, sq,
                                         start=(t == 0), stop=(t == KT - 1))
                s_row = normt.tile([1, n], f32, name="s_row")
                for c in range(NCH):
                    nc.scalar.activation(s_row[:, c * 512:(c + 1) * 512], ssq[c],
                                         AF.Sqrt, scale=1.0 / D)
                nc.vector.tensor_scalar_max(s_row, s_row, EPS)
                last_rs_inst = nc.vector.reciprocal(s_row, s_row)
                if dbg:
                    nc.sync.dma_start(out=dbg_d["drs"][:], in_=s_row)

                i1 = nc.sync.dma_start(out=cos_sb, in_=cos_d[:])
                i2 = nc.sync.dma_start(out=sin_sb, in_=sin_d[:])
                for i_ in (i1, i2):
                    add_dep(i_.ins, last_rs_inst.ins, True, "gate rot DMAs after norm")
                # fold rms scale into the rotary multipliers (reads bcast psum)
                for c in range(NCH):
                    bc = pbc.tile([P, 512], f32, name=f"bc{c}", tag="bc")
                    nc.tensor.matmul(bc, ones_row, s_row[:, c * 512:(c + 1) * 512],
                                     start=True, stop=True)
                    sl = slice(c * 512, (c + 1) * 512)
                    nc.vector.tensor_mul(cos_sb[:, sl], cos_sb[:, sl], bc)
                    nc.vector.tensor_mul(sin_sb[:, sl], sin_sb[:, sl], bc)
                    for tb in range(4):
                        tk = c * 4 + tb
                        dg = sqp.tile([P, P], f32, name=f"dg_{tk}", tag="dg")
                        nc.vector.tensor_mul(dg, bc[:, tb * P:(tb + 1) * P], ident_sb)
                        nc.vector.reduce_sum(rs_col[:, tk:tk + 1], dg,
                                             axis=mybir.AxisListType.X)

            with tc.tile_pool(name="pproj", bufs=3, space="PSUM") as pp, \
                 tc.tile_pool(name="ppv", bufs=3, space="PSUM") as ppv:
                rot_tail = []
                with tc.tile_pool(name="rotu", bufs=2) as rotu:
                    for base, wmain, wrot, nm_ in ((qT, wq, wqr, "q"), (kT, wk, wkr, "k")):
                        for c in range(NCH):
                            sl = slice(c * 512, (c + 1) * 512)
                            pss = []
                            for m in range(2):
                                ps = pp.tile([P, 512], f32,
                                             name=f"pp{nm_}{m}_{c}", tag="pp")
                                for t in range(KT):
                                    nc.tensor.matmul(
                                        ps, wmain[t][:, m * P:(m + 1) * P],
                                        x_sb[t][:, sl],
                                        start=(t == 0), stop=(t == KT - 1))
                                pss.append(ps)
                            for m in range(2):
                                nc.vector.tensor_mul(base[m][:, sl], pss[m],
                                                     cos_sb[:, sl])
                            for m in range(2):
                                psr = pp.tile([P, 512], f32,
                                              name=f"pp{nm_}r{m}_{c}", tag="pp")
                                for t in range(KT):
                                    nc.tensor.matmul(
                                        psr, wrot[t][:, m * P:(m + 1) * P],
                                        x_sb[t][:, sl],
                                        start=(t == 0), stop=(t == KT - 1))
                                u = rotu.tile([P, 512], f32,
                                              name=f"u_{nm_}{m}_{c}", tag="u")
                                nc.vector.tensor_mul(u, psr, sin_sb[:, sl])
                                rot_tail.append(
                                    nc.vector.tensor_add(base[m][:, sl],
                                                         base[m][:, sl], u))
                for tk in range(NTOK):
                    ps = ppv.tile([P, HPC * DH], f32, name=f"ppv_{tk}", tag="ppv")
                    for t in range(KT):
                        nc.tensor.matmul(ps, x_sb[t][:, tk * P:(tk + 1) * P], wv[t],
                                         start=(t == 0), stop=(t == KT - 1))
                    vv = v_sb[tk].rearrange("p (h c) -> p h c", h=HPC)
                    nc.vector.tensor_scalar_mul(
                        vv[:, :, 0:DH], ps.rearrange("p (h c) -> p h c", h=HPC),
                        rs_col[:, tk:tk + 1])
                    for hh in range(HPC):
                        nc.vector.tensor_copy(vv[:, hh, DH:DH + 1], ones_col)

        # wo loads once early-phase-A SBUF pressure has relaxed
        wo_sb = [wop.tile([P, D], mdt, name=f"wo{m}", tag=f"wo{m}") for m in range(2)]
        for m in range(2):
            iw = nc.sync.dma_start(out=wo_sb[m], in_=wo_d[m * P:(m + 1) * P, :])
            add_dep(iw.ins, rot_tail[-1].ins, True, "gate wo pool after rotary")

        # ---- attention + interleaved output projection ----
        with tc.tile_pool(name="ep", bufs=2) as ep, \
             tc.tile_pool(name="rbp", bufs=2) as rbp, \
             tc.tile_pool(name="bcdp", bufs=2) as bcdp, \
             tc.tile_pool(name="outsb", bufs=3) as osb, \
             tc.tile_pool(name="psim", bufs=1, space="PSUM") as psim, \
             tc.tile_pool(name="pmix", bufs=4, space="PSUM") as pmix:
            for qb in range(NQB):
                nkt = 4 * qb + 4
                qsl = slice(qb * 512, (qb + 1) * 512)
                attn = [late.tile([P, 512], mdt, name=f"attn{pr}_{qb}",
                                  tag=f"attn{pr}") for pr in range(2)]
                for pr in range(2):
                    pvh = [pmix.tile([DH + 1, 512], f32, name=f"pv_{pr}_{qb}_{h2}",
                                     tag="b512") for h2 in range(2)]
                    for g in range((nkt + 1) // 2):
                        kts = [z for z in (2 * g, 2 * g + 1) if z < nkt]
                        w_ = 512 * len(kts)
                        sims = [psim.tile([P, w_], f32, name=f"s{h2}_{pr}_{qb}_{g}",
                                          tag=f"sim{h2}") for h2 in range(2)]
                        for i, kt_ in enumerate(kts):
                            for h2 in range(2):
                                nc.tensor.matmul(
                                    sims[h2][:, i * 512:(i + 1) * 512],
                                    kT[pr][64 * h2:64 * h2 + 64, kt_ * P:(kt_ + 1) * P],
                                    qT[pr][64 * h2:64 * h2 + 64, qsl],
                                    start=True, stop=True, tile_position=(64 * h2, 0))
                        for i, kt_ in enumerate(kts):
                            d = kt_ - 4 * qb
                            for h2 in range(2):
                                if d >= 0:
                                    sl = sims[h2][:, i * 512 + d * P:i * 512 + (d + 1) * P]
                                    nc.vector.tensor_tensor(sl, sl, tri_sb, OP.add)
                                if use_kmask:
                                    sl = sims[h2][:, i * 512:(i + 1) * 512]
                                    nc.vector.tensor_scalar_add(sl, sl,
                                                                km_sb[:, kt_:kt_ + 1])
                        Es = [ep.tile([P, w_], mdt, name=f"E{h2}_{pr}_{qb}_{g}",
                                      tag=f"E{h2}") for h2 in range(2)]
                        for h2 in range(2):
                            nc.scalar.activation(Es[h2], sims[h2], AF.Exp)
                        for i, kt_ in enumerate(kts):
                            lo = max(0, kt_ - 4 * qb) * P
                            for h2 in range(2):
                                hh = 2 * pr + h2
                                nc.tensor.matmul(
                                    pvh[h2][:, lo:512],
                                    v_sb[kt_][:, (DH + 1) * hh:(DH + 1) * hh + DH + 1],
                                    Es[h2][:, i * 512 + lo:(i + 1) * 512],
                                    start=(kt_ == 0), stop=(kt_ == nkt - 1),
                                    skip_group_check=True)
                    for h2 in range(2):
                        if dbg:
                            nc.vector.tensor_copy(den_sb[2 * pr + h2][:, qsl],
                                                  pvh[h2][DH:DH + 1, :])
                        rb = rbp.tile([1, 512], f32, name=f"rb_{pr}_{qb}_{h2}", tag="rb")
                        nc.vector.reciprocal(rb, pvh[h2][DH:DH + 1, :])
                        bcd = bcdp.tile([DH, 512], f32, name=f"bcd_{pr}_{qb}_{h2}",
                                        tag="bcd")
                        bps = pmix.tile([DH, 512], f32, name=f"bps_{pr}_{qb}_{h2}",
                                        tag="b512")
                        nc.tensor.matmul(bps, ones_row[:, 0:DH], rb,
                                         start=True, stop=True)
                        nc.scalar.copy(bcd, bps)
                        nc.vector.tensor_tensor(
                            attn[pr][64 * h2:64 * h2 + 64, :], pvh[h2][0:DH, :],
                            bcd, OP.mult)
                # output projection for this q-block's token rows
                for tk in range(4 * qb, 4 * qb + 4):
                    tkl = tk - 4 * qb
                    for c2 in range(D // 512):
                        po = pmix.tile([P, 512], f32, name=f"po_{tk}_{c2}", tag="b512")
                        for m in range(2):
                            nc.tensor.matmul(po, attn[m][:, tkl * P:(tkl + 1) * P],
                                             wo_sb[m][:, c2 * 512:(c2 + 1) * 512],
                                             start=(m == 0), stop=(m == 1))
                        ob = osb.tile([P, 512], f32, name=f"ob_{tk}_{c2}", tag="ob")
                        nc.vector.tensor_copy(ob, po)
                        nc.sync.dma_start(
                            out=out_d[tk * P:(tk + 1) * P, c2 * 512:(c2 + 1) * 512],
                            in_=ob)
            if dbg:
                nc.sync.dma_start(out=dbg_d["dqT0"][:], in_=qT[0])
                nc.sync.dma_start(out=dbg_d["dqT1"][:], in_=qT[1])
                nc.sync.dma_start(out=dbg_d["dkT0"][:], in_=kT[0])
                nc.sync.dma_start(out=dbg_d["dv0"][:], in_=v_sb[0])

    nc.compile()
    return nc


# ---------------------------------------------------------------- host side

def np_dt(mm_dt):
    import ml_dtypes
    return {"f32": np.float32, "f32r": np.float32, "bf16": ml_dtypes.bfloat16}[mm_dt]


def make_core_inputs(x, mask, pos_emb, g, Wq, Wkv, Wo, core, n, mm_dt="f32r"):
    ndt = np_dt(mm_dt)
    b = core // 4
    h0 = (core % 4) * HPC
    scale = DH ** -0.5
    gW = Wq * g[:, None]
    gKV = Wkv * g[:, None]
    cols = slice(h0 * DH, (h0 + HPC) * DH)
    wq = gW[:, cols] * scale
    Wk_full = gKV[:, :D]
    Wv_full = gKV[:, D:]
    wk = Wk_full[:, cols]
    wv = Wv_full[:, cols]

    def rot_cols(W):
        # [h0r | 0 | h1r | 0, h2r | 0 | h3r | 0]: u tiles land aligned with qT
        out = np.zeros((D, 2 * P), dtype=W.dtype)
        for h in range(HPC):
            src = W[:, (h0 + h) * DH:(h0 + h) * DH + DH]
            base = h * DH
            out[:, base:base + 16] = -src[:, 16:32]
            out[:, base + 16:base + 32] = src[:, 0:16]
        return out

    wqr = rot_cols(gW) * scale
    wkr = rot_cols(Wk_full)
    wo = Wo[cols, :]

    cosf = np.cos(pos_emb.T).astype(np.float32)
    sinf = np.sin(pos_emb.T).astype(np.float32)
    cos128 = np.ones((P, n), np.float32)
    cos128[0:ROT] = cosf
    cos128[DH:DH + ROT] = cosf
    sin128 = np.zeros((P, n), np.float32)
    sin128[0:ROT] = sinf
    sin128[DH:DH + ROT] = sinf
    tri = np.where(np.arange(P)[:, None] <= np.arange(P)[None, :], 0.0, NEG
                   ).astype(np.float32)

    ins = {
        "xT": np.ascontiguousarray(x[b].T).astype(ndt),
        "wq": wq.astype(ndt), "wk": wk.astype(ndt), "wv": wv.astype(ndt),
        "wqr": wqr.astype(ndt), "wkr": wkr.astype(ndt), "wo": wo.astype(ndt),
        "cos128": cos128, "sin128": sin128, "tri": tri,
        "ident": np.eye(P, dtype=np.float32),
    }
    if not mask.all():
        km = np.where(mask[b], 0.0, NEG).astype(np.float32)
        ins["kmask"] = np.ascontiguousarray(km.reshape(n // P, P).T)
    return ins


# ---------------------------------------------------------------- runner

import os
import jax


def _run_per_device(nc, in_maps, core_ids):
    """Run the same Bass program independently on each visible device."""
    from concourse.bass2jax import (_bass_exec_p, install_neuronx_cc_hook,
                                    partition_id_tensor)
    install_neuronx_cc_hook()
    partition_name = nc.partition_id_tensor.name if nc.partition_id_tensor else None
    in_names, out_names, out_avals, zero_outs = [], [], [], []
    for alloc in nc.m.functions[0].allocations:
        if not isinstance(alloc, mybir.MemoryLocationSet):
            continue
        name = alloc.memorylocations[0].name
        if alloc.kind == "ExternalInput":
            if name != partition_name:
                in_names.append(name)
        elif alloc.kind == "ExternalOutput":
            out_names.append(name)
            shape = tuple(alloc.tensor_shape)
            dtype = mybir.dt.np(alloc.dtype)
            out_avals.append(jax.core.ShapedArray(shape, dtype))
            zero_outs.append(np.zeros(shape, dtype))
    n_params = len(in_names)
    all_in_names = list(in_names) + list(out_names)
    if partition_name is not None:
        all_in_names.append(partition_name)
    donate = tuple(range(n_params, n_params + len(out_names)))

    def _body(*args):
        operands = list(args)
        if partition_name is not None:
            operands.append(partition_id_tensor())
        outs = _bass_exec_p.bind(
            *operands, out_avals=tuple(out_avals), in_names=tuple(all_in_names),
            out_names=tuple(out_names), lowering_input_output_aliases=(),
            sim_require_finite=True, sim_require_nnan=True, nc=nc)
        return tuple(outs)

    fn = jax.jit(_body, donate_argnums=donate, keep_unused=True)
    futures = []
    for c, in_map in zip(core_ids, in_maps):
        dev = jax.devices()[c]
        args = [jax.device_put(np.asarray(in_map[nm]), dev) for nm in in_names]
        zz = [jax.device_put(z, dev) for z in zero_outs]
        futures.append(fn(*args, *zz))
    return [{nm: np.asarray(a) for nm, a in zip(out_names, f)} for f in futures]


_PROGRAM_CACHE = {}


def kernel(**inputs):
    os.environ.setdefault("NEURON_COMPILE_CACHE_URL", "/tmp/neuron_cache_kernel")
    x = np.asarray(inputs["x"], dtype=np.float32)
    mask = np.asarray(inputs["mask"]).astype(bool)
    pos_emb = np.asarray(inputs["pos_emb"], dtype=np.float32)
    g = np.asarray(inputs["g"], dtype=np.float32)
    Wq = np.asarray(inputs["Wq"], dtype=np.float32)
    Wkv = np.asarray(inputs["Wkv"], dtype=np.float32)
    Wo = np.asarray(inputs["Wo"], dtype=np.float32)
    bo = np.asarray(inputs["bo"], dtype=np.float32)
    b, n, _ = x.shape
    assert (b, n) == (2, 2048), (b, n)
    mm_dt = "f32r"
    use_km = not bool(mask.all())
    key = (n, mm_dt, use_km)
    if key not in _PROGRAM_CACHE:
        _PROGRAM_CACHE[key] = build_program(n=n, mm_dt=mm_dt, use_kmask=use_km)
    nc = _PROGRAM_CACHE[key]
    core_ids = list(range(8))
    in_maps = [make_core_inputs(x, mask, pos_emb, g, Wq, Wkv, Wo, c, n, mm_dt)
               for c in core_ids]
    results = _run_per_device(nc, in_maps, core_ids)
    out = np.zeros((b, n, D), np.float32)
    for c in core_ids:
        out[c // 4] += results[c]["out"]
    out += bo[None, None, :]
    return out



# revision 10
# speedup vs baseline: 1.5502x; 1.5502x over previous
"""Sharded causal attention kernel for trn2, v3.

Sharding: 8 cores = 2 batches x 4 head-groups (4 heads each).
v3 vs v2:
  - all SBUF tensors bf16 (DVE 2x, half DMA); PSUM stays f32
  - software-pipelined: attention for q-block qb runs one chunk behind the
    projections, emission interleaved at key-tile granularity so PE always
    has ready work while ACT churns exp
  - compact rotary weights [D,128] (no zero columns)
  - rs_col via PE transposes of ssq (no diag-extract trick)
  - pv with v-as-moving (65-row matmuls), per-partition softmax normalize,
    DMA-xbar transpose of attn, then out projection
"""

from contextlib import ExitStack

import numpy as np

import concourse.bass as bass
import concourse.mybir as mybir
import concourse.tile as tile
from concourse import bacc
from concourse.bass import _add_dep_helper as add_dep

f32 = mybir.dt.float32
f32r = mybir.dt.float32r
bf16 = mybir.dt.bfloat16
AF = mybir.ActivationFunctionType
OP = mybir.AluOpType

D = 1024
HPC = 4          # heads per core
DH = 64
ROT = 32
P = 128
NEG = -1e30


def build_program(n=2048, use_kmask=False):
    KT = D // P          # 8 contraction tiles
    NCH = n // 512       # 4 token chunks
    NTOK = n // P        # 16 token tiles
    nc = bacc.Bacc("TRN2", target_bir_lowering=False, debug=False)

    def din(name, shape, dt_):
        return nc.dram_tensor(name, shape, dt_, kind="ExternalInput")

    # host packs x/weights t-major so each is one DMA into a [128, ...] tile
    xT_d = din("xT", [P, KT * n], bf16)
    wq_d = din("wq", [P, KT * HPC * DH], bf16)
    wk_d = din("wk", [P, KT * HPC * DH], bf16)
    wv_d = din("wv", [P, KT * HPC * DH], bf16)
    wqr_d = din("wqr", [P, KT * P], bf16)          # compact rot cols [4 heads x 32]
    wkr_d = din("wkr", [P, KT * P], bf16)
    wo_d = din("wo", [HPC * DH, D], bf16)
    cos_d = din("cos128", [P, n], bf16)            # qT-aligned: rows 0:32,64:96 cos
    sin_d = din("sinc", [P, n], bf16)              # compact: row 32h+j = sin_j
    tri_d = din("tri", [P, P], f32)
    id_d = din("ident", [P, P], bf16)
    km_d = din("kmask", [P, NTOK], f32) if use_kmask else None
    out_d = nc.dram_tensor("out", [n, D], f32, kind="ExternalOutput")

    with tile.TileContext(nc) as tc, ExitStack() as top:
        persist = top.enter_context(tc.tile_pool(name="persist", bufs=1))
        ones_bf = persist.tile([P, 1], bf16, name="ones_bf")
        nc.vector.memset(ones_bf, 1.0)
        ones_row_f = persist.tile([1, P], f32, name="ones_row_f")
        nc.vector.memset(ones_row_f, 1.0)
        ones_row = persist.tile([1, P], f32r, name="ones_row")
        nc.vector.tensor_copy(ones_row, ones_row_f)
        tri_sb = persist.tile([P, P], f32, name="tri_sb")
        ident_sb = persist.tile([P, P], bf16, name="ident_sb")
        km_sb = persist.tile([P, NTOK], f32, name="km_sb") if use_kmask else None

        big = top.enter_context(tc.tile_pool(name="big", bufs=1))
        # x loaded chunk-major: one DMA brings all KT contraction tiles for a
        # 512-token chunk, so chunk-0 compute starts after ~1/4 of the x bytes
        x_all = big.tile([P, KT * n], bf16, name="x_all")
        x_allv = x_all.rearrange("p (t n) -> p t n", t=KT)
        xT_dv = xT_d.rearrange("p (t n) -> p t n", t=KT)
        x_sb = [x_all[:, t * n:(t + 1) * n] for t in range(KT)]
        wq_sb = big.tile([P, KT * HPC * DH], bf16, name="wq")
        wk_sb = big.tile([P, KT * HPC * DH], bf16, name="wk")
        wv_sb = big.tile([P, KT * HPC * DH], bf16, name="wv")
        wqr_sb = big.tile([P, KT * P], bf16, name="wqr")
        wkr_sb = big.tile([P, KT * P], bf16, name="wkr")
        cos_sb = big.tile([P, n], bf16, name="cos_sb")
        sin_sb = big.tile([P, n], bf16, name="sin_sb")
        wo_sb = [big.tile([P, D], bf16, name=f"wo{m}") for m in range(2)]
        # DMA issue order = single-queue service order: schedule each input
        # just before its first consumer needs it
        nc.sync.dma_start(out=x_allv[:, 0:2, 0:512], in_=xT_dv[:, 0:2, 0:512])
        nc.sync.dma_start(out=x_allv[:, 2:4, 0:512], in_=xT_dv[:, 2:4, 0:512])
        nc.sync.dma_start(out=x_allv[:, 4:KT, 0:512], in_=xT_dv[:, 4:KT, 0:512])
        nc.sync.dma_start(out=ident_sb, in_=id_d[:])
        nc.sync.dma_start(out=wq_sb, in_=wq_d[:])
        nc.sync.dma_start(out=wk_sb, in_=wk_d[:])
        nc.sync.dma_start(out=x_allv[:, :, 512:1024], in_=xT_dv[:, :, 512:1024])
        nc.sync.dma_start(out=wqr_sb, in_=wqr_d[:])
        nc.sync.dma_start(out=wkr_sb, in_=wkr_d[:])
        nc.sync.dma_start(out=cos_sb, in_=cos_d[:])
        nc.sync.dma_start(out=sin_sb, in_=sin_d[:])
        nc.sync.dma_start(out=x_allv[:, :, 1024:1536], in_=xT_dv[:, :, 1024:1536])
        nc.sync.dma_start(out=wv_sb, in_=wv_d[:])
        nc.sync.dma_start(out=x_allv[:, :, 1536:2048], in_=xT_dv[:, :, 1536:2048])
        nc.sync.dma_start(out=tri_sb, in_=tri_d[:])
        for m in range(2):
            nc.sync.dma_start(out=wo_sb[m], in_=wo_d[m * P:(m + 1) * P, :])
        if use_kmask:
            nc.sync.dma_start(out=km_sb, in_=km_d[:])
        wq = [wq_sb[:, t * HPC * DH:(t + 1) * HPC * DH] for t in range(KT)]
        wk = [wk_sb[:, t * HPC * DH:(t + 1) * HPC * DH] for t in range(KT)]
        wv = [wv_sb[:, t * HPC * DH:(t + 1) * HPC * DH] for t in range(KT)]
        wqr = [wqr_sb[:, t * P:(t + 1) * P] for t in range(KT)]
        wkr = [wkr_sb[:, t * P:(t + 1) * P] for t in range(KT)]

        qkv = top.enter_context(tc.tile_pool(name="qkv", bufs=1))
        qT = [qkv.tile([P, n], bf16, name=f"qT{m}", tag=f"qT{m}") for m in range(2)]
        kT = [qkv.tile([P, n], bf16, name=f"kT{m}", tag=f"kT{m}") for m in range(2)]
        v_sb = [qkv.tile([P, HPC * (DH + 1)], bf16, name=f"v{tk}", tag=f"v{tk}")
                for tk in range(NTOK)]
        rs_col = qkv.tile([P, NTOK], f32, name="rs_col")
        s_row = qkv.tile([1, n], f32r, name="s_row")
        ssq_sb = qkv.tile([1, n], bf16, name="ssq_sb")

        # PSUM budget (8 banks): pa (prelude + projections) 2,
        # pb (attention ppv accumulators + out-proj po) 2, psim 4
        pa = top.enter_context(tc.tile_pool(name="pa", bufs=2, space="PSUM"))
        pb = top.enter_context(tc.tile_pool(name="pb", bufs=1, space="PSUM"))
        psim = top.enter_context(tc.tile_pool(name="psim", bufs=1, space="PSUM"))
        sqp = top.enter_context(tc.tile_pool(name="sqp", bufs=2))
        rotu = top.enter_context(tc.tile_pool(name="rotu", bufs=2))
        esp = top.enter_context(tc.tile_pool(name="esp", bufs=1))
        atp = top.enter_context(tc.tile_pool(name="atp", bufs=2))
        obp = top.enter_context(tc.tile_pool(name="obp", bufs=2))
        rcp = top.enter_context(tc.tile_pool(name="rcp", bufs=2))

        # ---------------- phase A emitters (chunk c), as a piece list ----
        prelude_acts = []  # Sqrt instructions that must precede the first Exp

        def prelude(c):
            """rmsnorm stats for chunk c; all ACT Sqrt/Copy happen before the
            first Exp so the activation table loads exactly twice."""
            csl = slice(c * 512, (c + 1) * 512)
            ssq = pb.tile([1, 512], f32, name=f"ssq{c}", tag="ppv")
            for t in range(KT):
                sq = sqp.tile([P, 512], bf16, name=f"sq{t}_{c}", tag="sq")
                # spread squares across DVE/ACT/Pool (Square is in every act
                # table set; Pool is SBUF-only) to keep startup queues short
                if c == 0:
                    nc.vector.tensor_mul(sq, x_sb[t][:, csl], x_sb[t][:, csl])
                elif c == 2:
                    nc.scalar.activation(sq, x_sb[t][:, csl], AF.Square)
                else:
                    nc.gpsimd.tensor_mul(sq, x_sb[t][:, csl], x_sb[t][:, csl])
                nc.tensor.matmul(ssq, ones_bf, sq,
                                 start=(t == 0), stop=(t == KT - 1))
            # sqrt(ssq/D) then reciprocal (row form for bc broadcast)
            my_acts = []
            my_acts.append(
                nc.scalar.activation(s_row[:, csl], ssq, AF.Sqrt, scale=1.0 / D))
            with nc.allow_low_precision(reason="f32r is f32-width"):
                nc.vector.reciprocal(s_row[:, csl], s_row[:, csl])
            my_acts.append(
                nc.scalar.activation(ssq_sb[:, csl], ssq, AF.Copy))
            # rs_col tiles via PE transpose of ssq (bf16: f32r transpose is
            # broken in walrus codegen)
            rst = pb.tile([P, 8], bf16, name=f"rst{c}", tag="ppv")
            rstv = rst.rearrange("p (a b) -> p a b", b=2)
            for tb in range(4):
                tk = c * 4 + tb
                # even columns only: PSUM accesses must be 4-byte aligned
                nc.tensor.transpose(rstv[:, tb, 0:1],
                                    ssq_sb[:, tk * P:(tk + 1) * P],
                                    ones_bf[0:1, 0:1])
            my_acts.append(
                nc.scalar.activation(rs_col[:, c * 4:c * 4 + 4], rstv[:, :, 0],
                                     AF.Sqrt, scale=1.0 / D))
            if c < NCH - 1:
                # c3's stats come late (Pool squares); gating every exp on it
                # would stall attention — let it float and eat one table reload
                prelude_acts.extend(my_acts)
            nc.vector.reciprocal(rs_col[:, c * 4:c * 4 + 4],
                                 rs_col[:, c * 4:c * 4 + 4])
            # broadcast rs over partitions, fold into cos/sin
            bc = pb.tile([P, 512], f32, name=f"bc{c}", tag="ppv")
            nc.tensor.matmul(bc, ones_row, s_row[:, csl],
                             start=True, stop=True)
            nc.vector.tensor_mul(cos_sb[:, csl], cos_sb[:, csl], bc)
            nc.vector.tensor_mul(sin_sb[:, csl], sin_sb[:, csl], bc)

        def qk_pieces(c, which):
            csl = slice(c * 512, (c + 1) * 512)
            base, wmain, wrot, nm_ = ((qT, wq, wqr, "q") if which == "q"
                                      else (kT, wk, wkr, "k"))

            def p_m(m):
                ps = pa.tile([P, 512], f32, name=f"p{nm_}{m}_{c}", tag="pa")
                for t in range(KT):
                    nc.tensor.matmul(ps, wmain[t][:, m * P:(m + 1) * P],
                                     x_sb[t][:, csl],
                                     start=(t == 0), stop=(t == KT - 1))
                nc.vector.tensor_mul(base[m][:, csl], ps, cos_sb[:, csl])

            def p_rot():
                # psr partition layout (wqr col order [h0|h2|h1|h3]):
                # m=0 reads rows 0:96 (h0,-,h1), m=1 rows 32:128 (h2,-,h3);
                # u tiles land base-aligned with qT rot rows {0:32, 64:96} so
                # the SBUF-SBUF adds have equal base partitions (hw rule)
                psr = pa.tile([P, 512], f32, name=f"p{nm_}r_{c}", tag="pa")
                for t in range(KT):
                    nc.tensor.matmul(psr, wrot[t], x_sb[t][:, csl],
                                     start=(t == 0), stop=(t == KT - 1))
                for m in range(2):
                    u = rotu.tile([P, 512], bf16, name=f"u_{nm_}{m}_{c}", tag="u")
                    if m == 0:
                        nc.vector.tensor_mul(u[0:96, :], psr[0:96, :],
                                             sin_sb[0:96, csl])
                    else:
                        # aligned partition windows: <=32 parts from base 32/96
                        nc.vector.tensor_mul(u[0:32, :], psr[32:64, :],
                                             sin_sb[0:32, csl])
                        nc.vector.tensor_mul(u[64:96, :], psr[96:128, :],
                                             sin_sb[64:96, csl])
                    for h2 in range(2):
                        bsl = base[m][64 * h2:64 * h2 + 32, csl]
                        usl = u[64 * h2:64 * h2 + 32, :]
                        if c == 0:
                            nc.vector.tensor_tensor(bsl, bsl, usl, OP.add)
                        else:
                            nc.gpsimd.tensor_tensor(bsl, bsl, usl, OP.add)

            return [lambda: p_m(0), lambda: p_m(1), p_rot]

        def v_pieces(c):
            out = []
            for tb in range(4):
                tk = c * 4 + tb

                def p_v(tk=tk):
                    pv = pa.tile([P, HPC * DH], f32, name=f"pv_{tk}", tag="pa")
                    for t in range(KT):
                        nc.tensor.matmul(pv, x_sb[t][:, tk * P:(tk + 1) * P],
                                         wv[t], start=(t == 0), stop=(t == KT - 1))
                    vv = v_sb[tk].rearrange("p (h c2) -> p h c2", h=HPC)
                    nc.vector.tensor_scalar_mul(
                        vv[:, :, 0:DH], pv.rearrange("p (h c2) -> p h c2", h=HPC),
                        rs_col[:, tk:tk + 1])
                    for hh in range(HPC):
                        nc.gpsimd.tensor_copy(vv[:, hh, DH:DH + 1], ones_bf)

                out.append(p_v)
            return out

        # ---------------- attention emitter for q-block qb -----------------
        def emit_attention(qb, pieces):
            """pieces: phase-A closures spread evenly across the kt loop so PE
            has ready work while ACT churns exp."""
            nkt = 4 * qb + 4
            qsl = slice(qb * 512, (qb + 1) * 512)
            Es = {}
            # piece i fires after kt slot floor(i * nkt / npieces)
            slots = [[] for _ in range(nkt)]
            for i, p in enumerate(pieces):
                slots[min(nkt - 1, i * nkt // max(1, len(pieces)))].append(p)

            for kt in range(nkt):
                d = kt - 4 * qb
                for pr in range(2):
                    sim = psim.tile([P, 1024], f32, name=f"s{pr}_{qb}_{kt}",
                                    tag=f"sim{pr}")
                    for h2 in range(2):
                        nc.tensor.matmul(
                            sim[:, 512 * h2:512 * h2 + 512],
                            kT[pr][64 * h2:64 * h2 + 64, kt * P:(kt + 1) * P],
                            qT[pr][64 * h2:64 * h2 + 64, qsl],
                            start=True, stop=True, tile_position=(64 * h2, 0))
                    if d >= 0:
                        for h2 in range(2):
                            sl = sim[:, 512 * h2 + d * P:512 * h2 + (d + 1) * P]
                            nc.vector.tensor_tensor(sl, sl, tri_sb, OP.add)
                    if use_kmask:
                        for h2 in range(2):
                            sl = sim[:, 512 * h2:512 * h2 + 512]
                            nc.vector.tensor_scalar_add(sl, sl, km_sb[:, kt:kt + 1])
                    E = esp.tile([P, 1024], bf16, name=f"E{pr}_{kt}",
                                 tag=f"E{pr}_{kt}")
                    exps = []
                    if d >= 1:
                        for h2 in range(2):
                            sl = slice(512 * h2 + d * P, 512 * h2 + 512)
                            exps.append(nc.scalar.activation(E[:, sl], sim[:, sl],
                                                             AF.Exp))
                    else:
                        exps.append(nc.scalar.activation(E, sim, AF.Exp))
                    # keep every Sqrt before every Exp in the ACT stream: the
                    # scheduler otherwise interleaves them and forces repeated
                    # activation-table reloads
                    for e in exps:
                        for pa_i in prelude_acts:
                            add_dep(e.ins, pa_i.ins, True, "sqrt before exp")
                    Es[(pr, kt)] = E
                for p in slots[kt]:
                    p()
                if d >= 0:
                    # q-tile tb == d is complete: pv + normalize + out-proj
                    tb = d
                    qt = 4 * qb + tb
                    ppv = pb.tile([P, HPC * (DH + 1)], f32, name=f"ppv_{qt}",
                                  tag="ppv")
                    # one accumulation group at a time per bank: interleaved
                    # start/stop groups in a shared bank drop contributions
                    for pr in range(2):
                        for h2 in range(2):
                            hh = 2 * pr + h2
                            for kt2 in range(qt + 1):
                                nc.tensor.matmul(
                                    ppv[:, 65 * hh:65 * hh + 65],
                                    Es[(pr, kt2)][:, 512 * h2 + tb * P:
                                                  512 * h2 + (tb + 1) * P],
                                    v_sb[kt2][:, 65 * hh:65 * hh + 65],
                                    start=(kt2 == 0), stop=(kt2 == qt),
                                    skip_group_check=True)
                    rc = rcp.tile([P, HPC], f32, name=f"rc_{qt}", tag="rc")
                    pvw = ppv.rearrange("p (h c2) -> p h c2", c2=DH + 1)
                    nc.vector.reciprocal(rc, pvw[:, :, DH])
                    at = atp.tile([P, HPC * DH], bf16, name=f"at_{qt}", tag="at")
                    for hh in range(HPC):
                        nc.vector.tensor_scalar_mul(
                            at[:, DH * hh:DH * hh + DH],
                            ppv[:, 65 * hh:65 * hh + DH], rc[:, hh:hh + 1])
                    # transpose [tok, dims] -> [dims, tok] on PE, stage via Pool
                    tr = pb.tile([P, 2 * P], bf16, name=f"tr_{qt}", tag="po")
                    for m in range(2):
                        nc.tensor.transpose(tr[:, P * m:P * m + P],
                                            at[:, P * m:P * m + P], ident_sb)
                    atT = atp.tile([P, 2 * P], bf16, name=f"atT_{qt}", tag="atT")
                    nc.vector.tensor_copy(atT, tr)
                    for c2 in range(2):
                        po = pb.tile([P, 512], f32, name=f"po_{qt}_{c2}", tag="po")
                        for m in range(2):
                            nc.tensor.matmul(po, atT[:, P * m:P * m + P],
                                             wo_sb[m][:, 512 * c2:512 * c2 + 512],
                                             start=(m == 0), stop=(m == 1))
                        ob = obp.tile([P, 512], f32, name=f"ob_{qt}_{c2}", tag="ob")
                        if c2 == 0:
                            nc.vector.tensor_copy(ob, po)
                        else:
                            nc.scalar.activation(ob, po, AF.Copy)
                        nc.sync.dma_start(
                            out=out_d[qt * P:(qt + 1) * P,
                                      c2 * 512:(c2 + 1) * 512],
                            in_=ob)
        # ---------------- main schedule -----------------------------------
        # preludes + chunk-0 projections first (Sqrt table phase), then
        # attention blocks with later projection chunks as PE filler.
        # Filler assignment respects deps: attn(j) needs qT(j) done up front,
        # kT(j)/v(j) only by its diagonal key tiles (kt >= 4j).
        prelude(0)
        c0 = qk_pieces(0, "q") + qk_pieces(0, "k") + v_pieces(0)
        for i, p in enumerate(c0):
            p()
            if i in (1, 3, 5):
                prelude(i // 2 + 1)
        emit_attention(0, qk_pieces(1, "q") + qk_pieces(1, "k"))
        emit_attention(1, v_pieces(1) + qk_pieces(2, "q") + qk_pieces(2, "k"))
        emit_attention(2, v_pieces(2) + qk_pieces(3, "q"))
        emit_attention(3, qk_pieces(3, "k") + v_pieces(3))

    nc.compile()
    return nc


# ---------------------------------------------------------------- host side

def make_core_inputs(x, mask, pos_emb, g, Wq, Wkv, Wo, core, n):
    import ml_dtypes
    ndt = ml_dtypes.bfloat16
    b = core // 4
    h0 = (core % 4) * HPC
    scale = DH ** -0.5
    gW = Wq * g[:, None]
    gKV = Wkv * g[:, None]
    cols = slice(h0 * DH, (h0 + HPC) * DH)
    wq = gW[:, cols] * scale
    Wk_full = gKV[:, :D]
    wk = Wk_full[:, cols]
    wv = gKV[:, D:][:, cols]

    def rot_cols(W):
        # compact rotate-half sources; col-block order [h0|h2|h1|h3] so the
        # device-side u tiles land base-aligned with qT rot rows
        out = np.zeros((D, P), dtype=W.dtype)
        for b_, h in enumerate((0, 2, 1, 3)):
            src = W[:, (h0 + h) * DH:(h0 + h) * DH + DH]
            out[:, b_ * ROT:b_ * ROT + 16] = -src[:, 16:32]
            out[:, b_ * ROT + 16:b_ * ROT + 32] = src[:, 0:16]
        return out

    wqr = rot_cols(gW) * scale
    wkr = rot_cols(Wk_full)

    def pack_t(W):
        # [D, C] -> [128, KT*C] t-major
        C = W.shape[1]
        return np.ascontiguousarray(
            W.reshape(D // P, P, C).transpose(1, 0, 2).reshape(P, -1))

    cosf = np.cos(pos_emb.T).astype(np.float32)   # [32, n]
    sinf = np.sin(pos_emb.T).astype(np.float32)
    cos128 = np.ones((P, n), np.float32)
    cos128[0:ROT] = cosf
    cos128[DH:DH + ROT] = cosf
    sinc = np.zeros((P, n), np.float32)
    for h in range(HPC):
        sinc[h * ROT:(h + 1) * ROT] = sinf
    tri = np.where(np.arange(P)[:, None] <= np.arange(P)[None, :], 0.0, NEG
                   ).astype(np.float32)

    xT = np.ascontiguousarray(x[b].T)  # [D, n]
    ins = {
        "xT": pack_t(xT).astype(ndt),
        "wq": pack_t(wq).astype(ndt), "wk": pack_t(wk).astype(ndt),
        "wv": pack_t(wv).astype(ndt),
        "wqr": pack_t(wqr).astype(ndt), "wkr": pack_t(wkr).astype(ndt),
        "wo": Wo[cols, :].astype(ndt),
        "cos128": cos128.astype(ndt), "sinc": sinc.astype(ndt),
        "tri": tri, "ident": np.eye(P, dtype=ndt),
    }
    if not mask.all():
        km = np.where(mask[b], 0.0, NEG).astype(np.float32)
        ins["kmask"] = np.ascontiguousarray(km.reshape(n // P, P).T)
    return ins


# ---------------------------------------------------------------- runner

import os
import jax


def _run_per_device(nc, in_maps, core_ids):
    """Run the same Bass program independently on each visible device."""
    from concourse.bass2jax import (_bass_exec_p, install_neuronx_cc_hook,
                                    partition_id_tensor)
    install_neuronx_cc_hook()
    partition_name = nc.partition_id_tensor.name if nc.partition_id_tensor else None
    in_names, out_names, out_avals, zero_outs = [], [], [], []
    for alloc in nc.m.functions[0].allocations:
        if not isinstance(alloc, mybir.MemoryLocationSet):
            continue
        name = alloc.memorylocations[0].name
        if alloc.kind == "ExternalInput":
            if name != partition_name:
                in_names.append(name)
        elif alloc.kind == "ExternalOutput":
            out_names.append(name)
            shape = tuple(alloc.tensor_shape)
            dtype = mybir.dt.np(alloc.dtype)
            out_avals.append(jax.core.ShapedArray(shape, dtype))
            zero_outs.append(np.zeros(shape, dtype))
    n_params = len(in_names)
    all_in_names = list(in_names) + list(out_names)
    if partition_name is not None:
        all_in_names.append(partition_name)
    donate = tuple(range(n_params, n_params + len(out_names)))

    def _body(*args):
        operands = list(args)
        if partition_name is not None:
            operands.append(partition_id_tensor())
        outs = _bass_exec_p.bind(
            *operands, out_avals=tuple(out_avals), in_names=tuple(all_in_names),
            out_names=tuple(out_names), lowering_input_output_aliases=(),
            sim_require_finite=True, sim_require_nnan=True, nc=nc)
        return tuple(outs)

    fn = jax.jit(_body, donate_argnums=donate, keep_unused=True)
    futures = []
    for c, in_map in zip(core_ids, in_maps):
        dev = jax.devices()[c]
        args = [jax.device_put(np.asarray(in_map[nm]), dev) for nm in in_names]
        zz = [jax.device_put(z, dev) for z in zero_outs]
        futures.append(fn(*args, *zz))
    return [{nm: np.asarray(a) for nm, a in zip(out_names, f)} for f in futures]


_PROGRAM_CACHE = {}


def kernel(**inputs):
    os.environ.setdefault("NEURON_COMPILE_CACHE_URL", "/tmp/neuron_cache_kernel")
    x = np.asarray(inputs["x"], dtype=np.float32)
    mask = np.asarray(inputs["mask"]).astype(bool)
    pos_emb = np.asarray(inputs["pos_emb"], dtype=np.float32)
    g = np.asarray(inputs["g"], dtype=np.float32)
    Wq = np.asarray(inputs["Wq"], dtype=np.float32)
    Wkv = np.asarray(inputs["Wkv"], dtype=np.float32)
    Wo = np.asarray(inputs["Wo"], dtype=np.float32)
    bo = np.asarray(inputs["bo"], dtype=np.float32)
    b, n, _ = x.shape
    assert (b, n) == (2, 2048), (b, n)
    use_km = not bool(mask.all())
    key = (n, use_km)
    if key not in _PROGRAM_CACHE:
        _PROGRAM_CACHE[key] = build_program(n=n, use_kmask=use_km)
    nc = _PROGRAM_CACHE[key]
    core_ids = list(range(8))
    in_maps = [make_core_inputs(x, mask, pos_emb, g, Wq, Wkv, Wo, c, n)
               for c in core_ids]
    results = _run_per_device(nc, in_maps, core_ids)
    out = np.zeros((b, n, D), np.float32)
    for c in core_ids:
        out[c // 4] += results[c]["out"]
    out += bo[None, None, :]
    return out
